# revision 30
# baseline (speedup 1.0000x reference)
"""BERT self-attention on 8 Trainium2 NeuronCores (Bass/Tile).

Sharding: tensor-parallel over heads. Core c owns heads {2c, 2c+1}, i.e.
columns [128c, 128c+128) of Wq/Wk/Wv and of the output. Every core reads
the full hidden_states; no collectives — the host concatenates the 8
per-core [B*S, 128] outputs along the feature axis.

All matmul inputs are bf16 (host pre-converts X^T and the weight slices;
f32r gives no cost-model advantage over bf16 and bf16 halves DMA+SBUF).
Per-core pipeline (B=4, S=2048, D=1024, head_dim=64):

  projections (per batch b, 512-token chunk):
    Q^T/K^T [d', t] via 8-step bf16 matmuls (d' on partitions), bias add
    fused into the PSUM->SBUF copy; V computed DIRECTLY as [t, d'] (128-
    token blocks) so no PE transposes are needed anywhere. V_aug rows are
    pre-scaled by exp(mask) when the additive mask is nonzero, with the
    softmax-denominator "ones" column carrying exp(mask).

  attention (per b, head h, 512-wide q-chunk):
    S^T[k,q] = K Q^T in [128,2,512] PSUM groups; exp is SPLIT between the
    ACT engine (Exp activation, groups {0,1,3,4,6}) and the DVE (groups
    {2,5,7}) using a Schraudolph bitcast exp: int16(round(A*x+B)) bit-
    viewed as bf16 approximates e^(x/8) to ~1.8% RMS — one tensor_scalar
    (mult,add) instruction. es stays bf16.
    PV uses es as the STATIONARY operand ([k,128q] blocks) and V_aug
    [k, 66] bf16 as the MOVING operand, accumulating out[q, 66] over 16
    k-blocks — 66 moving rows per step instead of 512, and the result
    lands directly in [q, d] orientation (no output transposes). Column
    64 is the softmax denominator; DVE reciprocal+scale finishes.

  The projection work of batch b+1 is emitted in small pieces BETWEEN the
  score groups of batch b so the PE fills the gaps while ACT/DVE chew on
  exp backlogs.

V-bias is folded out entirely: softmax weights sum to 1, so out += bv is
exact and the host applies it after the gather.
"""

import numpy as np
import ml_dtypes

import concourse.bass as bass
import concourse.tile as tile
from concourse import bacc, mybir
from concourse.bass_utils import run_bass_kernel_spmd

B, S, D, H = 4, 2048, 1024, 16
DH = 64
N_CORES = 8
DPC = D // N_CORES  # 128 output dims (2 heads) per core
BS = B * S  # 8192

F32 = mybir.dt.float32
BF16 = mybir.dt.bfloat16
I16 = mybir.dt.int16

# Two-term Schraudolph bf16 exp on DVE:
#   es = bitcast_bf16(int16(A*x + B1)) + bitcast_bf16(int16(A*x + B2))
# approximates exp(x/8) to ~1.2% max / 0.7% RMS relative error. The "+" is
# folded into the PV matmul chain (both terms accumulate into PSUM), so a
# DVE exp group costs two tensor_scalar instructions and two extra PV steps.
# Constants fit for floor() int conversion semantics (DVE truncates).
import os as _os

SCH_A = 128.0 * np.log2(np.e) / 8.0  # 23.083120654223414
SCH_B1 = float(_os.environ.get("SCH_B1", 16256.0 - 167.9))
SCH_B2 = float(_os.environ.get("SCH_B2", 16256.0 - 105.4))

# exp-group assignment per (h,qch) segment, tuned so the PE stays the pacer:
# piece-rich segments run 7 groups on ACT + 1 on DVE (sum folded on DVE);
# pieceless (tail) segments run 6 on ACT + 2 on DVE with the sum folded
# into two extra PV accumulation steps.
DVE_GROUPS_STEADY = tuple(
    int(g) for g in _os.environ.get("DVE_STEADY", "5").split(",") if g != ""
)
DVE_GROUPS_TAIL = tuple(
    int(g) for g in _os.environ.get("DVE_TAIL", "2,6").split(",") if g != ""
)

_CACHE: dict = {}


def _build(use_mask: bool):
    nc = bacc.Bacc(
        "TRN2", target_bir_lowering=False, debug=False, enable_asserts=False
    )

    xtd = nc.dram_tensor("xt", [D, BS], BF16, kind="ExternalInput").ap()
    wq = nc.dram_tensor("wq", [D, DPC], BF16, kind="ExternalInput").ap()
    wk = nc.dram_tensor("wk", [D, DPC], BF16, kind="ExternalInput").ap()
    wv = nc.dram_tensor("wv", [D, DPC], BF16, kind="ExternalInput").ap()
    bq = nc.dram_tensor("bq", [DPC], F32, kind="ExternalInput").ap()
    bk = nc.dram_tensor("bk", [DPC], F32, kind="ExternalInput").ap()
    msk = nc.dram_tensor("msk", [B, S], F32, kind="ExternalInput").ap()
    out = nc.dram_tensor("out", [BS, DPC], F32, kind="ExternalOutput").ap()

    Exp = mybir.ActivationFunctionType.Exp
    MUL = mybir.AluOpType.mult
    ADD = mybir.AluOpType.add

    with tile.TileContext(nc) as tc:
        with (
            tc.tile_pool(name="consts", bufs=1) as consts,
            tc.tile_pool(name="p_xt", bufs=4) as p_xt,
            tc.tile_pool(name="p_qk", bufs=9) as p_qk,
            tc.tile_pool(name="p_v", bufs=9) as p_v,
            tc.tile_pool(name="p_es", bufs=18) as p_es,
            tc.tile_pool(name="p_fin", bufs=8) as p_fin,
            tc.tile_pool(name="ps_sp", bufs=3, space="PSUM") as ps_sp,
            tc.tile_pool(name="ps_acc", bufs=1, space="PSUM") as ps_acc,
            tc.tile_pool(name="ps_pv", bufs=1, space="PSUM") as ps_pv,
        ):
            # persistent PV accumulator: 7 sub-slots in ONE PSUM bank; the
            # Tile framework tracks subtile deps, so a chain only WARs the
            # readers of the chain 7 allocations back (~2 segments), which
            # keeps slow DVE fin-muls off the PE critical path
            pv_ring = ps_pv.tile([128, 7, DH + 2], F32, tag="pv")
            # ---- DMA order tuned for startup: wq first (first matmul
            # needs it), then the first X^T chunk in two halves (the q
            # chain starts as soon as the first half lands); wk/wv are
            # DMA'd lazily from the first k/v pieces, covered by the
            # preceding chains' PE time ----
            wq_sb = consts.tile([128, 8, DPC], BF16, tag="wq_sb")
            wk_sb = consts.tile([128, 8, DPC], BF16, tag="wk_sb")
            wv_sb = consts.tile([128, 8, DPC], BF16, tag="wv_sb")
            nc.sync.dma_start(out=wq_sb, in_=wq.rearrange("(cc p) d -> p cc d", p=128))
            xt0 = p_xt.tile([128, 8, 512], BF16, tag="xt")
            xt0_src = xtd[:, 0:512].rearrange("(cc p) t -> p cc t", p=128)
            nc.scalar.dma_start(out=xt0[:, 0:4, :], in_=xt0_src[:, 0:4, :])
            nc.scalar.dma_start(out=xt0[:, 4:8, :], in_=xt0_src[:, 4:8, :])

            bq_sb = consts.tile([128, 1], F32, tag="bq_sb")
            bk_sb = consts.tile([128, 1], F32, tag="bk_sb")
            nc.scalar.dma_start(out=bq_sb, in_=bq.rearrange("(p o) -> p o", o=1))
            nc.scalar.dma_start(out=bk_sb, in_=bk.rearrange("(p o) -> p o", o=1))

            if use_mask:
                m_sb = consts.tile([128, B, 16], F32, tag="m_sb")
                nc.sync.dma_start(
                    out=m_sb, in_=msk.rearrange("b (kb p) -> p b kb", p=128)
                )
                emask = consts.tile([128, B, 16], F32, tag="emask")
                nc.scalar.activation(emask, m_sb, Exp)

            # per-batch projection results (rotating pools)
            qT_t: dict = {}
            kT_t: dict = {}
            v_t: dict = {}

            def make_proj_pieces(b):
                """Emit-closures for batch b's projections: per 512-token
                chunk, piece A = xt DMA + Q chain, B = K chain, C = V
                chains (4 token-blocks, direct [t, d'] orientation).

                The prologue (b=0) runs its pieces back-to-back, so its
                PSUM accumulators come from the 3-deep (then idle) score
                pool; steady-state pieces are spaced by the attention
                interleave and share a single ring-1 bank."""
                pieces = []
                if b == 0:
                    acc_pool, acc_tag = ps_sp, "sp"
                else:
                    acc_pool, acc_tag = ps_acc, "acc"

                def tiles_for_chunk(b, tch):
                    qT = p_qk.tile([128, 512], BF16, tag="qT", name=f"qT{b}_{tch}")
                    kT = p_qk.tile([128, 512], BF16, tag="kT", name=f"kT{b}_{tch}")
                    v_sb = p_v.tile(
                        [128, 4, 2, DH + 2], BF16, tag="v_sb", name=f"v{b}_{tch}"
                    )
                    qT_t[(b, tch)] = qT
                    kT_t[(b, tch)] = kT
                    v_t[(b, tch)] = v_sb
                    return qT, kT, v_sb

                for tch in range(4):
                    t0 = b * S + tch * 512

                    def piece_a(b=b, tch=tch, t0=t0):
                        qT, kT, v_sb = tiles_for_chunk(b, tch)
                        if b == 0 and tch == 0:
                            xt = xt0
                        else:
                            xt = p_xt.tile([128, 8, 512], BF16, tag="xt")
                            nc.sync.dma_start(
                                out=xt,
                                in_=xtd[:, t0 : t0 + 512].rearrange(
                                    "(cc p) t -> p cc t", p=128
                                ),
                            )
                        qT_t[(b, tch, "xt")] = xt
                        acc = acc_pool.tile([128, 512], F32, tag=acc_tag, name="qacc")
                        for cc in range(8):
                            nc.tensor.matmul(
                                acc,
                                wq_sb[:, cc, :],
                                xt[:, cc, :],
                                start=(cc == 0),
                                stop=(cc == 7),
                            )
                        nc.vector.tensor_scalar_add(qT, acc, bq_sb)

                    def piece_b(b=b, tch=tch):
                        if b == 0 and tch == 0:
                            nc.scalar.dma_start(
                                out=wk_sb,
                                in_=wk.rearrange("(cc p) d -> p cc d", p=128),
                            )
                        xt = qT_t[(b, tch, "xt")]
                        kT = kT_t[(b, tch)]
                        acc = acc_pool.tile([128, 512], F32, tag=acc_tag, name="kacc")
                        for cc in range(8):
                            nc.tensor.matmul(
                                acc,
                                wk_sb[:, cc, :],
                                xt[:, cc, :],
                                start=(cc == 0),
                                stop=(cc == 7),
                            )
                        nc.vector.tensor_scalar_add(kT, acc, bk_sb)

                    def piece_c(b=b, tch=tch):
                        if b == 0 and tch == 0:
                            nc.scalar.dma_start(
                                out=wv_sb,
                                in_=wv.rearrange("(cc p) d -> p cc d", p=128),
                            )
                        xt = qT_t[(b, tch, "xt")]
                        v_sb = v_t[(b, tch)]
                        acc = acc_pool.tile([128, 4, 128], F32, tag=acc_tag, name="vacc")
                        for tb in range(4):
                            for cc in range(8):
                                nc.tensor.matmul(
                                    acc[:, tb, :],
                                    xt[:, cc, tb * 128 : (tb + 1) * 128],
                                    wv_sb[:, cc, :],
                                    start=(cc == 0),
                                    stop=(cc == 7),
                                )
                        for tb in range(4):
                            kb = tch * 4 + tb
                            if use_mask:
                                for h in range(2):
                                    nc.vector.tensor_scalar_mul(
                                        v_sb[:, tb, h, 0:DH],
                                        acc[:, tb, h * DH : (h + 1) * DH],
                                        emask[:, b, kb : kb + 1],
                                    )
                                    nc.vector.tensor_copy(
                                        v_sb[:, tb, h, DH : DH + 1],
                                        emask[:, b, kb : kb + 1],
                                    )
                                    nc.vector.tensor_copy(
                                        v_sb[:, tb, h, DH + 1 : DH + 2],
                                        emask[:, b, kb : kb + 1],
                                    )
                            else:
                                nc.vector.tensor_copy(
                                    v_sb[:, tb, :, 0:DH],
                                    acc[:, tb, :].rearrange("p (h d) -> p h d", h=2),
                                )
                        if not use_mask:
                            ones_dst = v_sb[:, :, :, DH : DH + 2]
                            nc.vector.memset(ones_dst, 1.0)

                    pieces.extend([piece_a, piece_b, piece_c])
                return pieces

            pv_slot = [0]
            pv_pending = []  # deferred PV blocks, emitted one segment late

            def flush_pv():
                while pv_pending:
                    emit_pv(*pv_pending.pop(0))

            def emit_scores(b, qch, h, dve_groups, dve_add, piece_cb):
                """Score matmuls + exp for one (h,qch) segment. Projection
                pieces pop after groups 1 and 5; the PREVIOUS segment's
                deferred PV block lands after group 3 — ready PE work that
                covers the ACT exp backlog mid-segment."""
                hp = h * DH
                es_g = []
                for g in range(8):
                    sp = ps_sp.tile([128, 2, 512], F32, tag="sp")
                    for j in range(2):
                        kb = 2 * g + j
                        nc.tensor.matmul(
                            sp[:, j, :],
                            kT_t[(b, kb // 4)][
                                hp : hp + DH,
                                (kb % 4) * 128 : (kb % 4 + 1) * 128,
                            ],
                            qT_t[(b, qch)][hp : hp + DH, :],
                            start=True,
                            stop=True,
                        )
                    es = p_es.tile([128, 2, 512], BF16, tag="es")
                    if g in dve_groups:
                        esa = p_es.tile([128, 2, 512], BF16, tag="est", bufs=8)
                        nc.vector.tensor_scalar(
                            esa.bitcast(I16), sp, SCH_A, SCH_B1, MUL, ADD
                        )
                        nc.vector.tensor_scalar(
                            es.bitcast(I16), sp, SCH_A, SCH_B2, MUL, ADD
                        )
                        if dve_add:
                            nc.vector.tensor_tensor(es, es, esa, ADD)
                            es_g.append((es,))
                        else:
                            es_g.append((es, esa))
                    else:
                        nc.scalar.activation(es, sp, Exp, scale=0.125)
                        es_g.append((es,))
                    if g in (1, 5):
                        piece_cb()
                    elif g == 3:
                        flush_pv()
                return es_g

            def emit_pv(b, qch, h, es_g):
                """PV + normalize + output DMA for one segment (emitted one
                segment late, so every dependency is comfortably old)."""
                hp = h * DH
                steps = [(es, kb) for kb in range(16) for es in es_g[kb // 2]]
                for qb in range(4):
                    pv = pv_ring[:, pv_slot[0] % 7, :]
                    pv_slot[0] += 1
                    for i, (es, kb) in enumerate(steps):
                        nc.tensor.matmul(
                            pv,
                            es[:, kb % 2, qb * 128 : (qb + 1) * 128],
                            v_t[(b, kb // 4)][:, kb % 4, h, :],
                            start=(i == 0),
                            stop=(i == len(steps) - 1),
                        )
                    rc = p_fin.tile([128, 1], F32, tag="rc")
                    nc.vector.reciprocal(rc, pv[:, DH : DH + 1])
                    fin = p_fin.tile([128, DH], F32, tag="fin")
                    nc.vector.tensor_scalar_mul(fin, pv[:, 0:DH], rc)
                    q0 = b * S + qch * 512 + qb * 128
                    nc.sync.dma_start(
                        out=out[q0 : q0 + 128, hp : hp + DH], in_=fin
                    )

            def emit_attention(b, pieces):
                """Attention for batch b, software-pipelined one segment
                deep: scores(seg i+1) are emitted before pv(seg i) so the
                PV chains never park on fresh exp results. Projection
                pieces of batch b+1 pop into the score-group slots. The
                batch's last PV block carries into the next batch."""
                n_slots = 16  # 2 piece slots per (h,qch) segment
                n_pieces = len(pieces)
                state = {"popped": 0, "slot": 0}

                def piece_cb():
                    if state["popped"] * n_slots < n_pieces * (state["slot"] + 1):
                        pieces[state["popped"]]()
                        state["popped"] += 1
                    state["slot"] += 1

                for qch in range(4):
                    for h in range(2):
                        seg_has_piece = state["popped"] < n_pieces
                        dve_groups = (
                            DVE_GROUPS_STEADY if seg_has_piece else DVE_GROUPS_TAIL
                        )
                        es_g = emit_scores(
                            b, qch, h, dve_groups, seg_has_piece, piece_cb
                        )
                        pv_pending.append((b, qch, h, es_g))
                del pieces[: state["popped"]]

            # prologue: batch 0 projections up front
            for piece in make_proj_pieces(0):
                piece()
            for b in range(B):
                pieces = make_proj_pieces(b + 1) if b + 1 < B else []
                emit_attention(b, pieces)
                for piece in pieces:  # leftovers (shouldn't happen)
                    piece()
            flush_pv()

    nc.compile()
    return nc


def _get_nc(use_mask: bool):
    if use_mask not in _CACHE:
        _CACHE[use_mask] = _build(use_mask)
    return _CACHE[use_mask]


def kernel(hidden_states, attention_mask, Wq, bq, Wk, bk, Wv, bv):
    bf = ml_dtypes.bfloat16
    xT = np.ascontiguousarray(
        np.asarray(hidden_states, dtype=np.float32).reshape(BS, D).T
    ).astype(bf)
    mask = np.ascontiguousarray(np.asarray(attention_mask, dtype=np.float32)).reshape(
        B, S
    )
    Wq = np.asarray(Wq, dtype=np.float32)
    Wk = np.asarray(Wk, dtype=np.float32)
    Wv = np.asarray(Wv, dtype=np.float32)
    bq = np.asarray(bq, dtype=np.float32)
    bk = np.asarray(bk, dtype=np.float32)
    bv = np.asarray(bv, dtype=np.float32)

    use_mask = bool(np.any(mask))
    nc = _get_nc(use_mask)

    in_maps = []
    for c in range(N_CORES):
        sl = slice(c * DPC, (c + 1) * DPC)
        in_maps.append(
            {
                "xt": xT,
                "wq": np.ascontiguousarray(Wq[:, sl]).astype(bf),
                "wk": np.ascontiguousarray(Wk[:, sl]).astype(bf),
                "wv": np.ascontiguousarray(Wv[:, sl]).astype(bf),
                "bq": np.ascontiguousarray(bq[sl]),
                "bk": np.ascontiguousarray(bk[sl]),
                "msk": mask,
            }
        )

    res = run_bass_kernel_spmd(nc, in_maps, core_ids=list(range(N_CORES)))
    parts = [res.results[c]["out"].reshape(B, S, DPC) for c in range(N_CORES)]
    full = np.concatenate(parts, axis=2)
    # V-bias folds through softmax exactly (weights sum to 1)
    if np.any(bv):
        full = full + bv[None, None, :]
    return full


# revision 31
# speedup vs baseline: 1.0426x; 1.0426x over previous
"""BERT self-attention on 8 Trainium2 NeuronCores (Bass/Tile).

Sharding: tensor-parallel over heads. Core c owns heads {2c, 2c+1}, i.e.
columns [128c, 128c+128) of Wq/Wk/Wv and of the output. Every core reads
the full hidden_states; no collectives — the host concatenates the 8
per-core [B*S, 128] outputs along the feature axis.

All matmul inputs are bf16 (host pre-converts X^T and the weight slices;
f32r gives no cost-model advantage over bf16 and bf16 halves DMA+SBUF).
Per-core pipeline (B=4, S=2048, D=1024, head_dim=64):

  projections (per batch b, 512-token chunk):
    Q^T/K^T [d', t] via 8-step bf16 matmuls (d' on partitions), bias add
    fused into the PSUM->SBUF copy; V computed DIRECTLY as [t, d'] (128-
    token blocks) so no PE transposes are needed anywhere. V_aug rows are
    pre-scaled by exp(mask) when the additive mask is nonzero, with the
    softmax-denominator "ones" column carrying exp(mask).

  attention (per b, head h, 512-wide q-chunk):
    S^T[k,q] = K Q^T in [128,2,512] PSUM groups; exp is SPLIT between the
    ACT engine (Exp activation, groups {0,1,3,4,6}) and the DVE (groups
    {2,5,7}) using a Schraudolph bitcast exp: int16(round(A*x+B)) bit-
    viewed as bf16 approximates e^(x/8) to ~1.8% RMS — one tensor_scalar
    (mult,add) instruction. es stays bf16.
    PV uses es as the STATIONARY operand ([k,128q] blocks) and V_aug
    [k, 66] bf16 as the MOVING operand, accumulating out[q, 66] over 16
    k-blocks — 66 moving rows per step instead of 512, and the result
    lands directly in [q, d] orientation (no output transposes). Column
    64 is the softmax denominator; DVE reciprocal+scale finishes.

  The projection work of batch b+1 is emitted in small pieces BETWEEN the
  score groups of batch b so the PE fills the gaps while ACT/DVE chew on
  exp backlogs.

V-bias is folded out entirely: softmax weights sum to 1, so out += bv is
exact and the host applies it after the gather.
"""

import numpy as np
import ml_dtypes

import concourse.bass as bass
import concourse.tile as tile
from concourse import bacc, mybir
from concourse.bass_utils import run_bass_kernel_spmd

B, S, D, H = 4, 2048, 1024, 16
DH = 64
N_CORES = 8
DPC = D // N_CORES  # 128 output dims (2 heads) per core
BS = B * S  # 8192

F32 = mybir.dt.float32
BF16 = mybir.dt.bfloat16
I16 = mybir.dt.int16

# Two-term Schraudolph bf16 exp on DVE:
#   es = bitcast_bf16(int16(A*x + B1)) + bitcast_bf16(int16(A*x + B2))
# approximates exp(x/8) to ~1.2% max / 0.7% RMS relative error. The "+" is
# folded into the PV matmul chain (both terms accumulate into PSUM), so a
# DVE exp group costs two tensor_scalar instructions and two extra PV steps.
# Constants fit for floor() int conversion semantics (DVE truncates).
import os as _os

SCH_A = 128.0 * np.log2(np.e) / 8.0  # 23.083120654223414
SCH_B1 = float(_os.environ.get("SCH_B1", 16256.0 - 167.9))
SCH_B2 = float(_os.environ.get("SCH_B2", 16256.0 - 105.4))

# exp-group assignment per (h,qch) segment, tuned so the PE stays the pacer:
# piece-rich segments run 7 groups on ACT + 1 on DVE (sum folded on DVE);
# pieceless (tail) segments run 6 on ACT + 2 on DVE with the sum folded
# into two extra PV accumulation steps.
DVE_GROUPS_STEADY = tuple(
    int(g) for g in _os.environ.get("DVE_STEADY", "5").split(",") if g != ""
)
DVE_GROUPS_TAIL = tuple(
    int(g) for g in _os.environ.get("DVE_TAIL", "2,6").split(",") if g != ""
)

_CACHE: dict = {}


def _build(use_mask: bool):
    nc = bacc.Bacc(
        "TRN2", target_bir_lowering=False, debug=False, enable_asserts=False
    )

    xtd = nc.dram_tensor("xt", [D, BS], BF16, kind="ExternalInput").ap()
    wq = nc.dram_tensor("wq", [D, DPC], BF16, kind="ExternalInput").ap()
    wk = nc.dram_tensor("wk", [D, DPC], BF16, kind="ExternalInput").ap()
    wv = nc.dram_tensor("wv", [D, DPC], BF16, kind="ExternalInput").ap()
    bq = nc.dram_tensor("bq", [DPC], F32, kind="ExternalInput").ap()
    bk = nc.dram_tensor("bk", [DPC], F32, kind="ExternalInput").ap()
    msk = nc.dram_tensor("msk", [B, S], F32, kind="ExternalInput").ap()
    out = nc.dram_tensor("out", [BS, DPC], F32, kind="ExternalOutput").ap()

    Exp = mybir.ActivationFunctionType.Exp
    MUL = mybir.AluOpType.mult
    ADD = mybir.AluOpType.add

    with tile.TileContext(nc) as tc:
        with (
            tc.tile_pool(name="consts", bufs=1) as consts,
            tc.tile_pool(name="p_xt", bufs=4) as p_xt,
            tc.tile_pool(name="p_qk", bufs=9) as p_qk,
            tc.tile_pool(name="p_v", bufs=9) as p_v,
            tc.tile_pool(name="p_es", bufs=18) as p_es,
            tc.tile_pool(name="p_fin", bufs=8) as p_fin,
            tc.tile_pool(name="ps_sp", bufs=3, space="PSUM") as ps_sp,
            tc.tile_pool(name="ps_acc", bufs=1, space="PSUM") as ps_acc,
            tc.tile_pool(name="ps_pv", bufs=1, space="PSUM") as ps_pv,
        ):
            # persistent PV accumulator: 7 sub-slots in ONE PSUM bank; the
            # Tile framework tracks subtile deps, so a chain only WARs the
            # readers of the chain 7 allocations back (~2 segments), which
            # keeps slow DVE fin-muls off the PE critical path
            pv_ring = ps_pv.tile([128, 7, DH + 2], F32, tag="pv")
            # ---- DMA order tuned for startup: wq first (first matmul
            # needs it), then the first X^T chunk in two halves (the q
            # chain starts as soon as the first half lands); wk/wv are
            # DMA'd lazily from the first k/v pieces, covered by the
            # preceding chains' PE time ----
            wq_sb = consts.tile([128, 8, DPC], BF16, tag="wq_sb")
            wk_sb = consts.tile([128, 8, DPC], BF16, tag="wk_sb")
            wv_sb = consts.tile([128, 8, DPC], BF16, tag="wv_sb")
            nc.sync.dma_start(out=wq_sb, in_=wq.rearrange("(cc p) d -> p cc d", p=128))
            xt0 = p_xt.tile([128, 8, 512], BF16, tag="xt")
            xt0_src = xtd[:, 0:512].rearrange("(cc p) t -> p cc t", p=128)
            nc.sync.dma_start(out=xt0[:, 0:4, :], in_=xt0_src[:, 0:4, :])
            nc.sync.dma_start(out=xt0[:, 4:8, :], in_=xt0_src[:, 4:8, :])
            nc.sync.dma_start(out=wk_sb, in_=wk.rearrange("(cc p) d -> p cc d", p=128))
            nc.sync.dma_start(out=wv_sb, in_=wv.rearrange("(cc p) d -> p cc d", p=128))

            bq_sb = consts.tile([128, 1], F32, tag="bq_sb")
            bk_sb = consts.tile([128, 1], F32, tag="bk_sb")
            nc.scalar.dma_start(out=bq_sb, in_=bq.rearrange("(p o) -> p o", o=1))
            nc.scalar.dma_start(out=bk_sb, in_=bk.rearrange("(p o) -> p o", o=1))

            if use_mask:
                m_sb = consts.tile([128, B, 16], F32, tag="m_sb")
                nc.scalar.dma_start(
                    out=m_sb, in_=msk.rearrange("b (kb p) -> p b kb", p=128)
                )
                emask = consts.tile([128, B, 16], F32, tag="emask")
                nc.scalar.activation(emask, m_sb, Exp)

            # per-batch projection results (rotating pools)
            qT_t: dict = {}
            kT_t: dict = {}
            v_t: dict = {}

            def make_proj_pieces(b):
                """Emit-closures for batch b's projections: per 512-token
                chunk, piece A = xt DMA + Q chain, B = K chain, C = V
                chains (4 token-blocks, direct [t, d'] orientation).

                The prologue (b=0) runs its pieces back-to-back, so its
                PSUM accumulators come from the 3-deep (then idle) score
                pool; steady-state pieces are spaced by the attention
                interleave and share a single ring-1 bank."""
                pieces = []
                if b == 0:
                    acc_pool, acc_tag = ps_sp, "sp"
                else:
                    acc_pool, acc_tag = ps_acc, "acc"

                def tiles_for_chunk(b, tch):
                    qT = p_qk.tile([128, 512], BF16, tag="qT", name=f"qT{b}_{tch}")
                    kT = p_qk.tile([128, 512], BF16, tag="kT", name=f"kT{b}_{tch}")
                    v_sb = p_v.tile(
                        [128, 4, 2, DH + 2], BF16, tag="v_sb", name=f"v{b}_{tch}"
                    )
                    qT_t[(b, tch)] = qT
                    kT_t[(b, tch)] = kT
                    v_t[(b, tch)] = v_sb
                    return qT, kT, v_sb

                for tch in range(4):
                    t0 = b * S + tch * 512

                    def piece_a(b=b, tch=tch, t0=t0):
                        qT, kT, v_sb = tiles_for_chunk(b, tch)
                        if b == 0 and tch == 0:
                            xt = xt0
                        else:
                            xt = p_xt.tile([128, 8, 512], BF16, tag="xt")
                            nc.sync.dma_start(
                                out=xt,
                                in_=xtd[:, t0 : t0 + 512].rearrange(
                                    "(cc p) t -> p cc t", p=128
                                ),
                            )
                        qT_t[(b, tch, "xt")] = xt
                        acc = acc_pool.tile([128, 512], F32, tag=acc_tag, name="qacc")
                        for cc in range(8):
                            nc.tensor.matmul(
                                acc,
                                wq_sb[:, cc, :],
                                xt[:, cc, :],
                                start=(cc == 0),
                                stop=(cc == 7),
                            )
                        nc.vector.tensor_scalar_add(qT, acc, bq_sb)

                    def piece_b(b=b, tch=tch):
                        xt = qT_t[(b, tch, "xt")]
                        kT = kT_t[(b, tch)]
                        acc = acc_pool.tile([128, 512], F32, tag=acc_tag, name="kacc")
                        for cc in range(8):
                            nc.tensor.matmul(
                                acc,
                                wk_sb[:, cc, :],
                                xt[:, cc, :],
                                start=(cc == 0),
                                stop=(cc == 7),
                            )
                        nc.vector.tensor_scalar_add(kT, acc, bk_sb)

                    def piece_c(b=b, tch=tch):
                        xt = qT_t[(b, tch, "xt")]
                        v_sb = v_t[(b, tch)]
                        acc = acc_pool.tile([128, 4, 128], F32, tag=acc_tag, name="vacc")
                        for tb in range(4):
                            for cc in range(8):
                                nc.tensor.matmul(
                                    acc[:, tb, :],
                                    xt[:, cc, tb * 128 : (tb + 1) * 128],
                                    wv_sb[:, cc, :],
                                    start=(cc == 0),
                                    stop=(cc == 7),
                                )
                        for tb in range(4):
                            kb = tch * 4 + tb
                            if use_mask:
                                for h in range(2):
                                    nc.vector.tensor_scalar_mul(
                                        v_sb[:, tb, h, 0:DH],
                                        acc[:, tb, h * DH : (h + 1) * DH],
                                        emask[:, b, kb : kb + 1],
                                    )
                                    nc.vector.tensor_copy(
                                        v_sb[:, tb, h, DH : DH + 1],
                                        emask[:, b, kb : kb + 1],
                                    )
                                    nc.vector.tensor_copy(
                                        v_sb[:, tb, h, DH + 1 : DH + 2],
                                        emask[:, b, kb : kb + 1],
                                    )
                            else:
                                nc.vector.tensor_copy(
                                    v_sb[:, tb, :, 0:DH],
                                    acc[:, tb, :].rearrange("p (h d) -> p h d", h=2),
                                )
                        if not use_mask:
                            ones_dst = v_sb[:, :, :, DH : DH + 2]
                            nc.vector.memset(ones_dst, 1.0)

                    pieces.extend([piece_a, piece_b, piece_c])
                return pieces

            pv_slot = [0]
            pv_pending = []  # deferred PV blocks, emitted one segment late

            def flush_pv():
                while pv_pending:
                    emit_pv(*pv_pending.pop(0))

            def emit_scores(b, qch, h, dve_groups, dve_add, piece_cb):
                """Score matmuls + exp for one (h,qch) segment. Projection
                pieces pop after groups 1 and 5; the PREVIOUS segment's
                deferred PV block lands after group 3 — ready PE work that
                covers the ACT exp backlog mid-segment."""
                hp = h * DH
                es_g = []
                for g in range(8):
                    sp = ps_sp.tile([128, 2, 512], F32, tag="sp")
                    for j in range(2):
                        kb = 2 * g + j
                        nc.tensor.matmul(
                            sp[:, j, :],
                            kT_t[(b, kb // 4)][
                                hp : hp + DH,
                                (kb % 4) * 128 : (kb % 4 + 1) * 128,
                            ],
                            qT_t[(b, qch)][hp : hp + DH, :],
                            start=True,
                            stop=True,
                        )
                    es = p_es.tile([128, 2, 512], BF16, tag="es")
                    if g in dve_groups:
                        esa = p_es.tile([128, 2, 512], BF16, tag="est", bufs=8)
                        nc.vector.tensor_scalar(
                            esa.bitcast(I16), sp, SCH_A, SCH_B1, MUL, ADD
                        )
                        nc.vector.tensor_scalar(
                            es.bitcast(I16), sp, SCH_A, SCH_B2, MUL, ADD
                        )
                        if dve_add:
                            nc.vector.tensor_tensor(es, es, esa, ADD)
                            es_g.append((es,))
                        else:
                            es_g.append((es, esa))
                    else:
                        nc.scalar.activation(es, sp, Exp, scale=0.125)
                        es_g.append((es,))
                    if g in (1, 5):
                        piece_cb()
                    elif g == 3:
                        flush_pv()
                return es_g

            def emit_pv(b, qch, h, es_g):
                """PV + normalize + output DMA for one segment (emitted one
                segment late, so every dependency is comfortably old)."""
                hp = h * DH
                steps = [(es, kb) for kb in range(16) for es in es_g[kb // 2]]
                for qb in range(4):
                    pv = pv_ring[:, pv_slot[0] % 7, :]
                    pv_slot[0] += 1
                    for i, (es, kb) in enumerate(steps):
                        nc.tensor.matmul(
                            pv,
                            es[:, kb % 2, qb * 128 : (qb + 1) * 128],
                            v_t[(b, kb // 4)][:, kb % 4, h, :],
                            start=(i == 0),
                            stop=(i == len(steps) - 1),
                        )
                    rc = p_fin.tile([128, 1], F32, tag="rc")
                    nc.vector.reciprocal(rc, pv[:, DH : DH + 1])
                    fin = p_fin.tile([128, DH], F32, tag="fin")
                    nc.vector.tensor_scalar_mul(fin, pv[:, 0:DH], rc)
                    q0 = b * S + qch * 512 + qb * 128
                    nc.sync.dma_start(
                        out=out[q0 : q0 + 128, hp : hp + DH], in_=fin
                    )

            def emit_attention(b, pieces):
                """Attention for batch b, software-pipelined one segment
                deep: scores(seg i+1) are emitted before pv(seg i) so the
                PV chains never park on fresh exp results. Projection
                pieces of batch b+1 pop into the score-group slots. The
                batch's last PV block carries into the next batch."""
                n_slots = 16  # 2 piece slots per (h,qch) segment
                n_pieces = len(pieces)
                state = {"popped": 0, "slot": 0}

                def piece_cb():
                    if state["popped"] * n_slots < n_pieces * (state["slot"] + 1):
                        pieces[state["popped"]]()
                        state["popped"] += 1
                    state["slot"] += 1

                for qch in range(4):
                    for h in range(2):
                        seg_has_piece = state["popped"] < n_pieces
                        dve_groups = (
                            DVE_GROUPS_STEADY if seg_has_piece else DVE_GROUPS_TAIL
                        )
                        es_g = emit_scores(
                            b, qch, h, dve_groups, seg_has_piece, piece_cb
                        )
                        pv_pending.append((b, qch, h, es_g))
                del pieces[: state["popped"]]

            # prologue: batch 0 projections up front
            for piece in make_proj_pieces(0):
                piece()
            for b in range(B):
                pieces = make_proj_pieces(b + 1) if b + 1 < B else []
                emit_attention(b, pieces)
                for piece in pieces:  # leftovers (shouldn't happen)
                    piece()
            flush_pv()

    nc.compile()
    return nc


def _get_nc(use_mask: bool):
    if use_mask not in _CACHE:
        _CACHE[use_mask] = _build(use_mask)
    return _CACHE[use_mask]


def kernel(hidden_states, attention_mask, Wq, bq, Wk, bk, Wv, bv):
    bf = ml_dtypes.bfloat16
    xT = np.ascontiguousarray(
        np.asarray(hidden_states, dtype=np.float32).reshape(BS, D).T
    ).astype(bf)
    mask = np.ascontiguousarray(np.asarray(attention_mask, dtype=np.float32)).reshape(
        B, S
    )
    Wq = np.asarray(Wq, dtype=np.float32)
    Wk = np.asarray(Wk, dtype=np.float32)
    Wv = np.asarray(Wv, dtype=np.float32)
    bq = np.asarray(bq, dtype=np.float32)
    bk = np.asarray(bk, dtype=np.float32)
    bv = np.asarray(bv, dtype=np.float32)

    use_mask = bool(np.any(mask))
    nc = _get_nc(use_mask)

    in_maps = []
    for c in range(N_CORES):
        sl = slice(c * DPC, (c + 1) * DPC)
        in_maps.append(
            {
                "xt": xT,
                "wq": np.ascontiguousarray(Wq[:, sl]).astype(bf),
                "wk": np.ascontiguousarray(Wk[:, sl]).astype(bf),
                "wv": np.ascontiguousarray(Wv[:, sl]).astype(bf),
                "bq": np.ascontiguousarray(bq[sl]),
                "bk": np.ascontiguousarray(bk[sl]),
                "msk": mask,
            }
        )

    res = run_bass_kernel_spmd(nc, in_maps, core_ids=list(range(N_CORES)))
    parts = [res.results[c]["out"].reshape(B, S, DPC) for c in range(N_CORES)]
    full = np.concatenate(parts, axis=2)
    # V-bias folds through softmax exactly (weights sum to 1)
    if np.any(bv):
        full = full + bv[None, None, :]
    return full
